# revision 30
# baseline (speedup 1.0000x reference)
"""MHGCN kernel for 8 Trainium2 NeuronCores — v2.

Row-shard A_stack [7,4096,4096] (bf16, host-cast) across 8 cores.
Phase 1 streams the 512x4096 strip in [128,2048] chunks and computes
  merged = sum_r w_r A_r        on the PE (7 scaled-identity matmuls
                                 PSUM-accumulated per 512-col bank)
  tanh-arg = sum_j Q_j E_j      on DVE/Pool with tensor_scalar (4x DVE
                                 perf mode) + tensor_tensor (2x)
    Q_j = R_j + (2/3)1[R_j>0],  E_j = sum_{o!=j} 0.6 M_jo R_o
  lt = merged + s*tanh(arg)     transposed on PE into FT (final_A^T)
AllToAll of merged is split into 4 per-row-tile collectives issued as
soon as each row tile finishes, so only the last ~15us is exposed.
Phase 2: FT += recv, fused [U1|V] = FT^T @ [Y1|G] matmuls, one bf16
AllGather of V, U2 = FT^T @ Y2, struct branch (rank-7, replicated),
combine + l2-normalize.
"""
import sys

sys.path.insert(0, "/opt/trn_rl_repo")

import numpy as np

import bass_rust
import concourse.bass as bass
import concourse.tile as tile
from concourse import mybir
from concourse.bass_utils import run_bass_kernel_spmd
from concourse.masks import make_identity
from concourse.vector_clock import ScopedClock

F32 = mybir.dt.float32
BF16 = mybir.dt.bfloat16
AF = mybir.ActivationFunctionType
OP = mybir.AluOpType

P = 128
N = 4096
NFEAT = 128
OUT = 64
NREL = 7
NCORES = 8
ROWS = N // NCORES        # 512 rows per core
NT = ROWS // P            # 4 row tiles per core
KT = N // P               # 32 k tiles
C = 2048                  # streaming column chunk
NCH = N // C              # 2 chunks per row tile
DST = ROWS                # alltoall chunk width (512)


def _patched_drain_and_barrier(self, tick_clock, wait_clock):
    # Stock Tile attaches every outstanding proc's sem wait to one Drain;
    # this walrus build caps sync waits per instruction, so split them
    # into single-wait drains.
    drain_inst = self.nc.sync.drain()
    wait_clock.add_sem_waits(
        drain_inst.ins, ScopedClock({None: tick_clock.global_clock})
    )
    si = drain_inst.ins.sync_info
    if si is not None and len(si.on_wait) > 1:
        waits = list(si.on_wait)
        si.on_wait = [waits[0]]
        for w in waits[1:]:
            extra = self.nc.sync.drain()
            extra.ins.sync_info = bass_rust.SyncInfo(on_wait=[w], on_update=[])
    self.nc.all_engine_barrier()
    assert self.sems is not None
    popped = self.nc._tile_sem_poison_stack.pop()
    assert popped is self._sem_poison
    self.nc.clear_and_free_semaphores(list(self.sems.allocated().values()))
    self.nc.all_engine_barrier()


tile.TileContext._drain_and_barrier = _patched_drain_and_barrier


def _split_multi_waits(nc, limit=1):
    """Walrus in this container caps sync-wait commands per instruction.
    Hoist all-but-`limit` waits of any instruction onto single-wait NoOps
    inserted just before it on the same engine queue."""
    cnt = 0
    for fn in nc.m.functions:
        for blk in fn.blocks:
            lst = list(blk.instructions)
            out = []
            changed = False
            for inst in lst:
                si = inst.sync_info
                if si is not None and len(si.on_wait) > limit:
                    waits = list(si.on_wait)
                    for w in waits[:-limit]:
                        n = bass_rust.InstNoOp(name=f"wsplit-{cnt}")
                        cnt += 1
                        n.engine = inst.engine
                        n.bass_nofuse = True
                        n.sync_info = bass_rust.SyncInfo(on_wait=[w],
                                                         on_update=[])
                        nc.register_instruction(n, overwrite=True)
                        out.append(n)
                    si.on_wait = waits[-limit:]
                    changed = True
                out.append(inst)
            if changed:
                blk.instructions = out
    return cnt


def _normalize(nc, pool, psum, x, out_dram, i):
    """l2-normalize rows of x [P, OUT] and DMA to out_dram[i*P:(i+1)*P]."""
    sq = pool.tile([P, OUT], F32, tag="nrm_sq")
    nrm = pool.tile([P, 1], F32, tag="nrm_n")
    nc.vector.tensor_tensor(sq[:], x[:], x[:], OP.mult)
    nc.vector.tensor_reduce(nrm[:], sq[:], mybir.AxisListType.X, OP.add)
    nr = pool.tile([P, 1], F32, tag="nrm_r")
    nc.scalar.activation(nr[:], nrm[:], AF.Sqrt)
    nc.vector.tensor_scalar(nr[:], nr[:], 1e-12, None, OP.max)
    ninv = pool.tile([P, 1], F32, tag="nrm_i")
    nc.vector.reciprocal(ninv[:], nr[:])
    y = pool.tile([P, OUT], F32, tag="nrm_y")
    nc.vector.tensor_scalar(y[:], x[:], ninv[:], None, OP.mult)
    nc.sync.dma_start(out=out_dram[i * P:(i + 1) * P, :], in_=y[:])


def build_nc():
    nc = bass.Bass()

    a_strip = nc.dram_tensor("a_strip", [NREL, ROWS, N], BF16, kind="ExternalInput")
    featT = nc.dram_tensor("featT", [NFEAT, N], BF16, kind="ExternalInput")
    encode = nc.dram_tensor("encode", [N, NREL], F32, kind="ExternalInput")
    enc_rows = nc.dram_tensor("enc_rows", [ROWS, NREL], F32, kind="ExternalInput")
    W1 = nc.dram_tensor("W1", [NFEAT, OUT], F32, kind="ExternalInput")
    W2 = nc.dram_tensor("W2", [OUT, OUT], F32, kind="ExternalInput")
    b1 = nc.dram_tensor("b1", [1, OUT], F32, kind="ExternalInput")
    b2 = nc.dram_tensor("b2", [1, OUT], F32, kind="ExternalInput")
    wb = nc.dram_tensor("wb", [1, NREL], F32, kind="ExternalInput")
    ri = nc.dram_tensor("ri", [1, 9], F32, kind="ExternalInput")
    s_ = nc.dram_tensor("s_", [1, 1], F32, kind="ExternalInput")
    sw = nc.dram_tensor("sw", [NREL, 1], F32, kind="ExternalInput")

    o_res = nc.dram_tensor("o_res", [ROWS, OUT], F32, kind="ExternalOutput")
    o_b1 = nc.dram_tensor("o_b1", [ROWS, OUT], F32, kind="ExternalOutput")
    o_b2 = nc.dram_tensor("o_b2", [ROWS, OUT], F32, kind="ExternalOutput")

    groups = [list(range(NCORES))]

    with tile.TileContext(nc) as tc:
        with (
            tc.tile_pool(name="persist", bufs=1) as pp,
            tc.tile_pool(name="dram", bufs=1, space="DRAM") as dpool,
        ):
            # ---- constants / small tensors ----
            ident = pp.tile([P, P], F32)
            make_identity(nc, ident)
            identb = pp.tile([P, P], BF16)
            nc.vector.tensor_copy(identb[:], ident[:])

            ones_1p = pp.tile([1, P], F32)
            nc.vector.memset(ones_1p[:], 1.0)

            # scalar staging: [0:7]=w_r, [7:16]=M flat, [16]=s
            sstage = pp.tile([1, 17], F32)
            nc.sync.dma_start(out=sstage[:, 0:NREL], in_=wb[:])
            nc.sync.dma_start(out=sstage[:, NREL:NREL + 9], in_=ri[:])
            nc.sync.dma_start(out=sstage[:, 16:17], in_=s_[:])

            W1t = pp.tile([NFEAT, OUT], F32)
            nc.sync.dma_start(out=W1t[:], in_=W1[:])
            W2t = pp.tile([OUT, OUT], F32)
            nc.sync.dma_start(out=W2t[:], in_=W2[:])
            b1st = pp.tile([1, OUT], F32)
            nc.sync.dma_start(out=b1st[:], in_=b1[:])
            b2st = pp.tile([1, OUT], F32)
            nc.sync.dma_start(out=b2st[:], in_=b2[:])
            swt = pp.tile([NREL, 1], F32)
            nc.sync.dma_start(out=swt[:], in_=sw[:])

            scal = pp.tile([P, 17], F32)
            b1b = pp.tile([P, OUT], F32)
            b2b = pp.tile([P, OUT], F32)
            with tc.tile_pool(name="ppsum", bufs=1, space="PSUM") as pps:
                pb = pps.tile([P, 17], F32, tag="pb")
                nc.tensor.matmul(pb[:], lhsT=ones_1p[:], rhs=sstage[:],
                                 start=True, stop=True)
                nc.vector.tensor_copy(scal[:], pb[:])
                pb1 = pps.tile([P, OUT], F32, tag="pb1")
                nc.tensor.matmul(pb1[:], lhsT=ones_1p[:], rhs=b1st[:],
                                 start=True, stop=True)
                nc.vector.tensor_copy(b1b[:], pb1[:])
                pb2 = pps.tile([P, OUT], F32, tag="pb2")
                nc.tensor.matmul(pb2[:], lhsT=ones_1p[:], rhs=b2st[:],
                                 start=True, stop=True)
                nc.vector.tensor_copy(b2b[:], pb2[:])

            # fp32 broadcast scalars (TensorScalarPtr requires fp32 scalars)
            scal04 = pp.tile([P, 9], F32)
            nc.vector.tensor_scalar(scal04[:], scal[:, NREL:NREL + 9], 0.4,
                                    None, OP.mult)

            def w_ap(r):
                return scal[:, r:r + 1]

            s_ap = scal[:, 16:17]

            def c04_ap(i, j):
                return scal04[:, 3 * i + j:3 * i + j + 1]

            # scaled identities for the PE merged accumulation
            identw = []
            for r in range(NREL):
                t = pp.tile([P, P], BF16, tag=f"idw{r}")
                nc.vector.tensor_scalar(t[:], identb[:], w_ap(r), None, OP.mult)
                identw.append(t)
            # s-scaled identity: folds final_A = merged + s*tanh into the
            # transpose accumulation
            sidentb = pp.tile([P, P], BF16)
            nc.vector.tensor_scalar(sidentb[:], identb[:], s_ap, None, OP.mult)

            # ---- persistent big tensors (bf16; PSUM accumulates fp32) ----
            FT = pp.tile([P, KT * ROWS], BF16)    # final_A^T: 32 k-tiles x [128, 512]
            YG = pp.tile([P, KT * 2 * OUT], BF16)  # [Y1 | G] per k-tile

            # ---- DRAM bounce buffers (per-row-tile split collectives) ----
            sendb = [dpool.tile([NCORES * P, DST], BF16, tag=f"snd{i}",
                                name=f"sendb{i}")
                     for i in range(NT)]
            recvb = [dpool.tile([NCORES * P, DST], BF16, tag=f"rcv{i}",
                                name=f"recvb{i}")
                     for i in range(NT)]
            agin = dpool.tile([ROWS, OUT], BF16)
            agout = dpool.tile([N, OUT], BF16, addr_space="Shared")

            # ---- prep: Y1 = feature @ W1 ; G = feature @ (W1 W2) ----
            with (
                tc.tile_pool(name="prep", bufs=1) as prep,
                tc.tile_pool(name="preppsum", bufs=2, space="PSUM") as prps,
            ):
                fbf = prep.tile([NFEAT, N], BF16)
                nc.sync.dma_start(out=fbf[:], in_=featT[:])
                W1b = pp.tile([NFEAT, OUT], BF16)
                nc.vector.tensor_copy(W1b[:], W1t[:])
                W2b = pp.tile([OUT, OUT], BF16)
                nc.vector.tensor_copy(W2b[:], W2t[:])
                # W12 = W1 @ W2 (via W1^T transpose), h = b1 @ W2
                pw1t = prps.tile([P, P], BF16, tag="prsm")
                nc.tensor.transpose(pw1t[:OUT, :NFEAT], W1b[:], identb[:])
                W1T = prep.tile([OUT, NFEAT], BF16)
                nc.vector.tensor_copy(W1T[:], pw1t[:OUT, :NFEAT])
                pw12 = prps.tile([NFEAT, OUT], F32, tag="prsm")
                nc.tensor.matmul(pw12[:], lhsT=W1T[:], rhs=W2b[:],
                                 start=True, stop=True)
                W12b = pp.tile([NFEAT, OUT], BF16)
                nc.vector.tensor_copy(W12b[:], pw12[:])
                b1v = prep.tile([OUT, 1], BF16)
                pb1t = prps.tile([OUT, 1], BF16, tag="prsm")
                b1bf = prep.tile([1, OUT], BF16)
                nc.vector.tensor_copy(b1bf[:], b1st[:])
                nc.tensor.transpose(pb1t[:], b1bf[:], identb[:1, :1])
                nc.vector.tensor_copy(b1v[:], pb1t[:])
                phh = prps.tile([1, OUT], F32, tag="prsm")
                nc.tensor.matmul(phh[:], lhsT=b1v[:], rhs=W2b[:],
                                 start=True, stop=True)
                hst = prep.tile([1, OUT], F32)
                nc.vector.tensor_copy(hst[:], phh[:])
                phb = prps.tile([P, OUT], F32, tag="prsm")
                nc.tensor.matmul(phb[:], lhsT=ones_1p[:], rhs=hst[:],
                                 start=True, stop=True)
                hbf = pp.tile([P, OUT], BF16)
                nc.vector.tensor_copy(hbf[:], phb[:])

                for kt in range(KT):
                    pm = prps.tile([P, 2 * OUT], F32, tag="y1p")
                    nc.tensor.matmul(pm[:, :OUT],
                                     lhsT=fbf[:, kt * P:(kt + 1) * P],
                                     rhs=W1b[:], start=True, stop=True)
                    nc.tensor.matmul(pm[:, OUT:],
                                     lhsT=fbf[:, kt * P:(kt + 1) * P],
                                     rhs=W12b[:], start=True, stop=True)
                    nc.vector.tensor_copy(
                        YG[:, kt * 2 * OUT:(kt + 1) * 2 * OUT], pm[:])

            # ---- phase 1: stream A row block ----
            with (
                tc.tile_pool(name="rstr", bufs=2) as prr,
                tc.tile_pool(name="istr", bufs=2) as pis,
                tc.tile_pool(name="mstr", bufs=2) as pms,
                tc.tile_pool(name="mpsum", bufs=1, space="PSUM") as mps,
                tc.tile_pool(name="tpsum", bufs=4, space="PSUM") as tps,
            ):
                # transposes/copies run one chunk behind so the DVE never
                # stalls on the tanh -> PE -> copy chain of the same chunk
                pending = []

                def flush_pending():
                    # FT is i-major: [:, (i*KT + kt)*P], so 4 consecutive
                    # k-tiles share one wide PSUM tile and one wide copy
                    for (pi, pq, pmbf, ptT) in pending:
                        pc0 = pq * C
                        for g in range(C // P // 4):
                            kt0 = pc0 // P + g * 4
                            pt1 = tps.tile([P, 4 * P], F32, tag="ptg")
                            for t4 in range(4):
                                t = g * 4 + t4
                                sl = pt1[:, t4 * P:(t4 + 1) * P]
                                nc.tensor.matmul(
                                    sl, lhsT=pmbf[:, t * P:(t + 1) * P],
                                    rhs=identb[:], start=True, stop=False)
                                nc.tensor.matmul(
                                    sl, lhsT=ptT[:, t * P:(t + 1) * P],
                                    rhs=sidentb[:], start=False, stop=True)
                            fsl = FT[:, (pi * KT + kt0) * P:
                                     (pi * KT + kt0 + 4) * P]
                            nc.vector.tensor_copy(fsl, pt1[:])
                    pending.clear()

                for i in range(NT):
                    for q in range(NCH):
                        c0 = q * C
                        rb = []
                        for r in range(NREL):
                            rt = prr.tile([P, C], BF16, tag=f"r{r}")
                            nc.sync.dma_start(
                                out=rt[:],
                                in_=a_strip[r, i * P:(i + 1) * P, c0:c0 + C])
                            rb.append(rt)

                        # merged on PE: psum[b] = sum_r w_r R_r[:, b*512:...]
                        mpt = []
                        for b in range(4):
                            mp = mps.tile([P, 512], F32, tag=f"mp{b}")
                            for r in range(NREL):
                                nc.tensor.matmul(
                                    mp[:], lhsT=identw[r][:],
                                    rhs=rb[r][:, b * 512:(b + 1) * 512],
                                    start=(r == 0), stop=(r == NREL - 1))
                            mpt.append(mp)
                        # cast merged to bf16 (GpSimd cannot read PSUM) + send
                        mbf = pms.tile([P, C], BF16, tag="mbf")
                        for b in range(4):
                            nc.vector.tensor_copy(
                                mbf[:, b * 512:(b + 1) * 512], mpt[b][:])
                            d = q * 4 + b
                            nc.sync.dma_start(
                                out=sendb[i][d * P:(d + 1) * P, :],
                                in_=mbf[:, b * 512:(b + 1) * 512])

                        # interaction: acc = sum_j Q_j E_j
                        #   Q_j = 1.5 R_j + sign(R_j), E_j = sum 0.4 M_jo R_o
                        acc = pis.tile([P, C], BF16, tag="acc")
                        for j in range(3):
                            o1, o2 = [x for x in range(3) if x != j]
                            ta = pis.tile([P, C], BF16, tag="ta")
                            nc.scalar.activation(ta[:], rb[o1][:], AF.Copy,
                                                 scale=c04_ap(j, o1))
                            tb = pis.tile([P, C], BF16, tag="tb")
                            if j == 0:
                                nc.scalar.activation(tb[:], rb[o2][:], AF.Copy,
                                                     scale=c04_ap(j, o2))
                            else:
                                nc.vector.tensor_scalar(tb[:], rb[o2][:],
                                                        c04_ap(j, o2), None,
                                                        OP.mult)
                            ej = pis.tile([P, C], BF16, tag="ej")
                            nc.vector.tensor_tensor(ej[:], ta[:], tb[:], OP.add)
                            sg = pis.tile([P, C], BF16, tag="sg")
                            nc.scalar.sign(sg[:], rb[j][:])
                            qs = pis.tile([P, C], BF16, tag="qs")
                            nc.vector.tensor_scalar(qs[:], rb[j][:], 1.5,
                                                    None, OP.mult)
                            qj = pis.tile([P, C], BF16, tag="qj")
                            nc.vector.tensor_tensor(qj[:], qs[:], sg[:], OP.add)
                            if j == 0:
                                nc.vector.tensor_tensor(acc[:], qj[:], ej[:],
                                                        OP.mult)
                            else:
                                pj = pis.tile([P, C], BF16, tag="pj")
                                nc.vector.tensor_tensor(pj[:], qj[:], ej[:],
                                                        OP.mult)
                                nc.gpsimd.tensor_tensor(acc[:], acc[:],
                                                        pj[:], OP.add)
                        tT = pis.tile([P, C], BF16, tag="tT")
                        nc.scalar.activation(tT[:], acc[:], AF.Tanh)

                        flush_pending()
                        pending.append((i, q, mbf, tT))

                    # row tile i complete -> alltoall its merged rows
                    # (sends only need mbf, which is never deferred)
                    nc.gpsimd.collective_compute(
                        "AllToAll", OP.bypass, replica_groups=groups,
                        ins=[sendb[i][:].opt()], outs=[recvb[i][:].opt()])
                flush_pending()

            # ---- phase 2 ----
            with (
                tc.tile_pool(name="post", bufs=1) as post,
                tc.tile_pool(name="rcvp", bufs=4) as prc,
                tc.tile_pool(name="postpsum", bufs=1, space="PSUM") as pops,
                tc.tile_pool(name="u1psum", bufs=2, space="PSUM") as pup,
                tc.tile_pool(name="u1vpsum", bufs=1, space="PSUM") as pvac,
            ):
                # ---- stripe-major: FT += recv#j, then V and U1 matmuls
                # for stripe j, so stripes 0-2 overlap the last alltoall.
                # V and U1 accumulate separately so the V path can finish
                # (and the allgather launch) before U1 wraps up. ----
                pV = [pvac.tile([P, OUT], F32, tag=f"pV{i}",
                                name=f"pV{i}") for i in range(NT)]
                U1sb = post.tile([P, NT * OUT], F32, name="U1sb")

                def stripe_adds(j):
                    for d in range(NCORES):
                        kt = d * NT + j
                        rt = prc.tile([P, DST], BF16, tag="rt")
                        nc.sync.dma_start(out=rt[:],
                                          in_=recvb[j][d * P:(d + 1) * P, :])
                        for i in range(NT):
                            fsl = FT[:, (i * KT + kt) * P:(i * KT + kt + 1) * P]
                            nc.vector.tensor_tensor(fsl, fsl,
                                                    rt[:, i * P:(i + 1) * P],
                                                    OP.add)

                def stripe_mm_v(j):
                    # V: one psum accumulation chain per row tile, held
                    # across all four stripes (one bank each)
                    for i in range(NT):
                        for d in range(NCORES):
                            kt = d * NT + j
                            nc.tensor.matmul(
                                pV[i][:],
                                lhsT=FT[:, (i * KT + kt) * P:
                                        (i * KT + kt + 1) * P],
                                rhs=YG[:, kt * 2 * OUT + OUT:
                                        (kt + 1) * 2 * OUT],
                                start=(j == 0 and d == 0),
                                stop=(j == NT - 1 and d == NCORES - 1))

                def stripe_mm_u(j):
                    # U1: per-stripe psum chain, accumulated into U1sb (SBUF)
                    for i in range(NT):
                        pu = pup.tile([P, OUT], F32, tag="pUji")
                        for d in range(NCORES):
                            kt = d * NT + j
                            nc.tensor.matmul(
                                pu[:],
                                lhsT=FT[:, (i * KT + kt) * P:
                                        (i * KT + kt + 1) * P],
                                rhs=YG[:, kt * 2 * OUT:kt * 2 * OUT + OUT],
                                start=(d == 0), stop=(d == NCORES - 1))
                        usl = U1sb[:, i * OUT:(i + 1) * OUT]
                        if j == 0:
                            nc.vector.tensor_tensor(usl, pu[:], b1b[:], OP.add)
                        else:
                            nc.vector.tensor_tensor(usl, usl, pu[:], OP.add)

                for j in range(NT - 1):
                    stripe_adds(j)
                    stripe_mm_v(j)
                    stripe_mm_u(j)

                # ---- struct branch (rank-7), overlaps alltoall tail ----
                encsb = post.tile([P, KT * NREL], F32)
                for kt in range(KT):
                    nc.sync.dma_start(out=encsb[:, kt * NREL:(kt + 1) * NREL],
                                      in_=encode[kt * P:(kt + 1) * P, :])
                encb = post.tile([P, KT * NREL], BF16)
                nc.vector.tensor_copy(encb[:], encsb[:])
                encT = post.tile([NREL, N], BF16)
                for kt in range(KT):
                    pte = pops.tile([P, P], BF16, tag="pp_tr")
                    nc.tensor.transpose(pte[:NREL, :],
                                        encb[:, kt * NREL:(kt + 1) * NREL],
                                        identb[:])
                    nc.vector.tensor_copy(encT[:, kt * P:(kt + 1) * P],
                                          pte[:NREL, :])
                encRsb = post.tile([P, NT * NREL], F32)
                for i in range(NT):
                    nc.sync.dma_start(out=encRsb[:, i * NREL:(i + 1) * NREL],
                                      in_=enc_rows[i * P:(i + 1) * P, :])
                encRb = post.tile([P, NT * NREL], BF16)
                nc.vector.tensor_copy(encRb[:], encRsb[:])
                encRT = post.tile([NREL, ROWS], BF16)
                for i in range(NT):
                    pte = pops.tile([P, P], BF16, tag="pp_tr")
                    nc.tensor.transpose(pte[:NREL, :],
                                        encRb[:, i * NREL:(i + 1) * NREL],
                                        identb[:])
                    nc.vector.tensor_copy(encRT[:, i * P:(i + 1) * P],
                                          pte[:NREL, :])

                # H1 = encode^T @ Y1, scaled by sw
                ph = pops.tile([NREL, OUT], F32, tag="pp_mm")
                for kt in range(KT):
                    nc.tensor.matmul(ph[:],
                                     lhsT=encb[:, kt * NREL:(kt + 1) * NREL],
                                     rhs=YG[:, kt * 2 * OUT:kt * 2 * OUT + OUT],
                                     start=(kt == 0), stop=(kt == KT - 1))
                H1p = post.tile([NREL, OUT], BF16)
                nc.scalar.activation(H1p[:], ph[:], AF.Copy, scale=swt[:])

                # U3 = encode @ H1p + b1 (full, replicated)
                U3sb = post.tile([P, KT * OUT], BF16)
                for kt in range(KT):
                    pm3 = pops.tile([P, OUT], F32, tag="pp_mm")
                    nc.tensor.matmul(pm3[:], lhsT=encT[:, kt * P:(kt + 1) * P],
                                     rhs=H1p[:], start=True, stop=True)
                    nc.vector.tensor_tensor(U3sb[:, kt * OUT:(kt + 1) * OUT],
                                            pm3[:], b1b[:], OP.add)

                # ---- final stripe: V path first so the allgather launches
                # as early as possible; U1 and struct part B fill its shadow
                stripe_adds(NT - 1)
                stripe_mm_v(NT - 1)
                for i in range(NT):
                    vt = post.tile([P, OUT], BF16, tag="vt", bufs=2)
                    nc.vector.tensor_copy(vt[:], pV[i][:])
                    nc.sync.dma_start(out=agin[i * P:(i + 1) * P, :], in_=vt[:])
                nc.gpsimd.collective_compute(
                    "AllGather", OP.bypass, replica_groups=groups,
                    ins=[agin[:].opt()], outs=[agout[:].opt()])
                stripe_mm_u(NT - 1)

                # G2 = (encode^T @ U3) @ W2, scaled by sw
                pg = pops.tile([NREL, OUT], F32, tag="pp_mm")
                for kt in range(KT):
                    nc.tensor.matmul(pg[:],
                                     lhsT=encb[:, kt * NREL:(kt + 1) * NREL],
                                     rhs=U3sb[:, kt * OUT:(kt + 1) * OUT],
                                     start=(kt == 0), stop=(kt == KT - 1))
                Gsb = post.tile([NREL, OUT], BF16)
                nc.vector.tensor_copy(Gsb[:], pg[:])
                pgt = pops.tile([OUT, NREL], BF16, tag="pp_tr")
                nc.tensor.transpose(pgt[:], Gsb[:], identb[:NREL, :NREL])
                GT = post.tile([OUT, NREL], BF16)
                nc.vector.tensor_copy(GT[:], pgt[:])
                pg2 = pops.tile([NREL, OUT], F32, tag="pp_mm")
                nc.tensor.matmul(pg2[:], lhsT=GT[:], rhs=W2b[:],
                                 start=True, stop=True)
                G2p = post.tile([NREL, OUT], BF16)
                nc.scalar.activation(G2p[:], pg2[:], AF.Copy, scale=swt[:])

                # U4 rows = enc_rows @ G2p + b2
                U4sb = post.tile([P, NT * OUT], F32)
                for i in range(NT):
                    pm4 = pops.tile([P, OUT], F32, tag="pp_mm")
                    nc.tensor.matmul(pm4[:], lhsT=encRT[:, i * P:(i + 1) * P],
                                     rhs=G2p[:], start=True, stop=True)
                    nc.vector.tensor_tensor(U4sb[:, i * OUT:(i + 1) * OUT],
                                            pm4[:], b2b[:], OP.add)

                # ---- Y2 = V_full + b1@W2 ----
                Y2 = post.tile([P, KT * OUT], BF16)
                for kt in range(KT):
                    vtk = post.tile([P, OUT], BF16, tag="vtk", bufs=4)
                    nc.sync.dma_start(out=vtk[:],
                                      in_=agout[kt * P:(kt + 1) * P, :])
                    nc.vector.tensor_tensor(Y2[:, kt * OUT:(kt + 1) * OUT],
                                            vtk[:], hbf[:], OP.add)

                # ---- layer 2: U2 = final_A @ Y2 + b2 ----
                U2sb = post.tile([P, NT * OUT], F32)
                for i in range(NT):
                    pm = pops.tile([P, OUT], F32, tag="pp_mm")
                    for kt in range(KT):
                        nc.tensor.matmul(
                            pm[:],
                            lhsT=FT[:, (i * KT + kt) * P:(i * KT + kt + 1) * P],
                            rhs=Y2[:, kt * OUT:(kt + 1) * OUT],
                            start=(kt == 0), stop=(kt == KT - 1))
                    nc.vector.tensor_tensor(U2sb[:, i * OUT:(i + 1) * OUT],
                                            pm[:], b2b[:], OP.add)

                # ---- combine + normalize + store ----
                for i in range(NT):
                    sl = slice(i * OUT, (i + 1) * OUT)
                    br1 = post.tile([P, OUT], F32, tag="br1", bufs=2)
                    nc.vector.tensor_tensor(br1[:], U1sb[:, sl], U2sb[:, sl],
                                            OP.add)
                    nc.vector.tensor_scalar(br1[:], br1[:], 0.5, None, OP.mult)
                    res = post.tile([P, OUT], F32, tag="res", bufs=2)
                    nc.vector.tensor_tensor(res[:], br1[:], U4sb[:, sl], OP.add)
                    nc.vector.tensor_scalar(res[:], res[:], 0.5, None, OP.mult)
                    _normalize(nc, post, pops, res, o_res, i)
                    _normalize(nc, post, pops, br1, o_b1, i)
                    u4 = post.tile([P, OUT], F32, tag="u4n", bufs=2)
                    nc.vector.tensor_copy(u4[:], U4sb[:, sl])
                    _normalize(nc, post, pops, u4, o_b2, i)

    _split_multi_waits(nc)
    return nc


_NC_CACHE = None


def get_nc():
    global _NC_CACHE
    if _NC_CACHE is None:
        _NC_CACHE = build_nc()
    return _NC_CACHE


def make_in_maps(feature, A_stack, encode, W1, b1, W2, b2, weight_b,
                 relation_interaction, interaction_strength, struct_weight):
    import ml_dtypes
    f32 = lambda x: np.ascontiguousarray(np.asarray(x, dtype=np.float32))
    featT = np.ascontiguousarray(
        np.asarray(feature, np.float32).T.astype(ml_dtypes.bfloat16))
    enc = f32(encode)
    common = dict(
        featT=featT,
        encode=enc,
        W1=f32(W1),
        W2=f32(W2),
        b1=f32(np.reshape(b1, (1, OUT))),
        b2=f32(np.reshape(b2, (1, OUT))),
        wb=f32(np.reshape(np.asarray(weight_b, np.float32)[:, 0], (1, NREL))),
        ri=f32(np.reshape(relation_interaction, (1, 9))),
        s_=f32(np.reshape(interaction_strength, (1, 1))),
        sw=f32(np.reshape(struct_weight, (NREL, 1))),
    )
    in_maps = []
    A = np.asarray(A_stack, np.float32).astype(ml_dtypes.bfloat16)
    for c in range(NCORES):
        rows = slice(c * ROWS, (c + 1) * ROWS)
        m = dict(common)
        m["a_strip"] = np.ascontiguousarray(A[:, rows, :])
        m["enc_rows"] = f32(enc[rows])
        in_maps.append(m)
    return in_maps


def run(inputs, trace=False, tmpdir=None):
    nc = get_nc()
    in_maps = make_in_maps(**inputs)
    kres = run_bass_kernel_spmd(nc, in_maps, list(range(NCORES)),
                                trace=trace, tmpdir=tmpdir)
    res = kres.results
    result = np.concatenate([res[c]["o_res"] for c in range(NCORES)], axis=0)
    branch1 = np.concatenate([res[c]["o_b1"] for c in range(NCORES)], axis=0)
    branch2 = np.concatenate([res[c]["o_b2"] for c in range(NCORES)], axis=0)
    return (result, branch1, branch2), kres


def kernel(**inputs):
    return run(inputs)[0]


# revision 31
# speedup vs baseline: 1.0484x; 1.0484x over previous
"""MHGCN kernel for 8 Trainium2 NeuronCores — v2.

Row-shard A_stack [7,4096,4096] (bf16, host-cast) across 8 cores.
Phase 1 streams the 512x4096 strip in [128,2048] chunks and computes
  merged = sum_r w_r A_r        on the PE (7 scaled-identity matmuls
                                 PSUM-accumulated per 512-col bank)
  tanh-arg = sum_j Q_j E_j      on DVE/Pool with tensor_scalar (4x DVE
                                 perf mode) + tensor_tensor (2x)
    Q_j = R_j + (2/3)1[R_j>0],  E_j = sum_{o!=j} 0.6 M_jo R_o
  lt = merged + s*tanh(arg)     transposed on PE into FT (final_A^T)
AllToAll of merged is split into 4 per-row-tile collectives issued as
soon as each row tile finishes, so only the last ~15us is exposed.
Phase 2: FT += recv, fused [U1|V] = FT^T @ [Y1|G] matmuls, one bf16
AllGather of V, U2 = FT^T @ Y2, struct branch (rank-7, replicated),
combine + l2-normalize.
"""
import sys

sys.path.insert(0, "/opt/trn_rl_repo")

import numpy as np

import bass_rust
import concourse.bass as bass
import concourse.tile as tile
from concourse import mybir
from concourse.bass_utils import run_bass_kernel_spmd
from concourse.masks import make_identity
from concourse.vector_clock import ScopedClock

F32 = mybir.dt.float32
BF16 = mybir.dt.bfloat16
AF = mybir.ActivationFunctionType
OP = mybir.AluOpType

P = 128
N = 4096
NFEAT = 128
OUT = 64
NREL = 7
NCORES = 8
ROWS = N // NCORES        # 512 rows per core
NT = ROWS // P            # 4 row tiles per core
KT = N // P               # 32 k tiles
C = 2048                  # streaming column chunk
NCH = N // C              # 2 chunks per row tile
DST = ROWS                # alltoall chunk width (512)


def _patched_drain_and_barrier(self, tick_clock, wait_clock):
    # Stock Tile attaches every outstanding proc's sem wait to one Drain;
    # this walrus build caps sync waits per instruction, so split them
    # into single-wait drains.
    drain_inst = self.nc.sync.drain()
    wait_clock.add_sem_waits(
        drain_inst.ins, ScopedClock({None: tick_clock.global_clock})
    )
    si = drain_inst.ins.sync_info
    if si is not None and len(si.on_wait) > 1:
        waits = list(si.on_wait)
        si.on_wait = [waits[0]]
        for w in waits[1:]:
            extra = self.nc.sync.drain()
            extra.ins.sync_info = bass_rust.SyncInfo(on_wait=[w], on_update=[])
    self.nc.all_engine_barrier()
    assert self.sems is not None
    popped = self.nc._tile_sem_poison_stack.pop()
    assert popped is self._sem_poison
    self.nc.clear_and_free_semaphores(list(self.sems.allocated().values()))
    self.nc.all_engine_barrier()


tile.TileContext._drain_and_barrier = _patched_drain_and_barrier


def _split_multi_waits(nc, limit=1):
    """Walrus in this container caps sync-wait commands per instruction.
    Hoist all-but-`limit` waits of any instruction onto single-wait NoOps
    inserted just before it on the same engine queue."""
    cnt = 0
    for fn in nc.m.functions:
        for blk in fn.blocks:
            lst = list(blk.instructions)
            out = []
            changed = False
            for inst in lst:
                si = inst.sync_info
                if si is not None and len(si.on_wait) > limit:
                    waits = list(si.on_wait)
                    for w in waits[:-limit]:
                        n = bass_rust.InstNoOp(name=f"wsplit-{cnt}")
                        cnt += 1
                        n.engine = inst.engine
                        n.bass_nofuse = True
                        n.sync_info = bass_rust.SyncInfo(on_wait=[w],
                                                         on_update=[])
                        nc.register_instruction(n, overwrite=True)
                        out.append(n)
                    si.on_wait = waits[-limit:]
                    changed = True
                out.append(inst)
            if changed:
                blk.instructions = out
    return cnt


def _normalize(nc, pool, psum, x, out_dram, i):
    """l2-normalize rows of x [P, OUT] and DMA to out_dram[i*P:(i+1)*P]."""
    sq = pool.tile([P, OUT], F32, tag="nrm_sq")
    nrm = pool.tile([P, 1], F32, tag="nrm_n")
    nc.vector.tensor_tensor(sq[:], x[:], x[:], OP.mult)
    nc.vector.tensor_reduce(nrm[:], sq[:], mybir.AxisListType.X, OP.add)
    nr = pool.tile([P, 1], F32, tag="nrm_r")
    nc.scalar.activation(nr[:], nrm[:], AF.Sqrt)
    nc.vector.tensor_scalar(nr[:], nr[:], 1e-12, None, OP.max)
    ninv = pool.tile([P, 1], F32, tag="nrm_i")
    nc.vector.reciprocal(ninv[:], nr[:])
    y = pool.tile([P, OUT], F32, tag="nrm_y")
    nc.vector.tensor_scalar(y[:], x[:], ninv[:], None, OP.mult)
    nc.sync.dma_start(out=out_dram[i * P:(i + 1) * P, :], in_=y[:])


def build_nc():
    nc = bass.Bass()

    a_strip = nc.dram_tensor("a_strip", [NREL, ROWS, N], BF16, kind="ExternalInput")
    featT = nc.dram_tensor("featT", [NFEAT, N], BF16, kind="ExternalInput")
    encode = nc.dram_tensor("encode", [N, NREL], F32, kind="ExternalInput")
    enc_rows = nc.dram_tensor("enc_rows", [ROWS, NREL], F32, kind="ExternalInput")
    W1 = nc.dram_tensor("W1", [NFEAT, OUT], F32, kind="ExternalInput")
    W2 = nc.dram_tensor("W2", [OUT, OUT], F32, kind="ExternalInput")
    b1 = nc.dram_tensor("b1", [1, OUT], F32, kind="ExternalInput")
    b2 = nc.dram_tensor("b2", [1, OUT], F32, kind="ExternalInput")
    wb = nc.dram_tensor("wb", [1, NREL], F32, kind="ExternalInput")
    ri = nc.dram_tensor("ri", [1, 9], F32, kind="ExternalInput")
    s_ = nc.dram_tensor("s_", [1, 1], F32, kind="ExternalInput")
    sw = nc.dram_tensor("sw", [NREL, 1], F32, kind="ExternalInput")

    o_res = nc.dram_tensor("o_res", [ROWS, OUT], F32, kind="ExternalOutput")
    o_b1 = nc.dram_tensor("o_b1", [ROWS, OUT], F32, kind="ExternalOutput")
    o_b2 = nc.dram_tensor("o_b2", [ROWS, OUT], F32, kind="ExternalOutput")

    groups = [list(range(NCORES))]

    with tile.TileContext(nc) as tc:
        with (
            tc.tile_pool(name="persist", bufs=1) as pp,
            tc.tile_pool(name="dram", bufs=1, space="DRAM") as dpool,
        ):
            # ---- constants / small tensors ----
            ident = pp.tile([P, P], F32)
            make_identity(nc, ident)
            identb = pp.tile([P, P], BF16)
            nc.vector.tensor_copy(identb[:], ident[:])

            ones_1p = pp.tile([1, P], F32)
            nc.vector.memset(ones_1p[:], 1.0)

            # scalar staging: [0:7]=w_r, [7:16]=M flat, [16]=s
            sstage = pp.tile([1, 17], F32)
            nc.sync.dma_start(out=sstage[:, 0:NREL], in_=wb[:])
            nc.sync.dma_start(out=sstage[:, NREL:NREL + 9], in_=ri[:])
            nc.sync.dma_start(out=sstage[:, 16:17], in_=s_[:])

            W1t = pp.tile([NFEAT, OUT], F32)
            nc.sync.dma_start(out=W1t[:], in_=W1[:])
            W2t = pp.tile([OUT, OUT], F32)
            nc.sync.dma_start(out=W2t[:], in_=W2[:])
            b1st = pp.tile([1, OUT], F32)
            nc.sync.dma_start(out=b1st[:], in_=b1[:])
            b2st = pp.tile([1, OUT], F32)
            nc.sync.dma_start(out=b2st[:], in_=b2[:])
            swt = pp.tile([NREL, 1], F32)
            nc.sync.dma_start(out=swt[:], in_=sw[:])

            scal = pp.tile([P, 17], F32)
            b1b = pp.tile([P, OUT], F32)
            b2b = pp.tile([P, OUT], F32)
            with tc.tile_pool(name="ppsum", bufs=1, space="PSUM") as pps:
                pb = pps.tile([P, 17], F32, tag="pb")
                nc.tensor.matmul(pb[:], lhsT=ones_1p[:], rhs=sstage[:],
                                 start=True, stop=True)
                nc.vector.tensor_copy(scal[:], pb[:])
                pb1 = pps.tile([P, OUT], F32, tag="pb1")
                nc.tensor.matmul(pb1[:], lhsT=ones_1p[:], rhs=b1st[:],
                                 start=True, stop=True)
                nc.vector.tensor_copy(b1b[:], pb1[:])
                pb2 = pps.tile([P, OUT], F32, tag="pb2")
                nc.tensor.matmul(pb2[:], lhsT=ones_1p[:], rhs=b2st[:],
                                 start=True, stop=True)
                nc.vector.tensor_copy(b2b[:], pb2[:])

            # fp32 broadcast scalars (TensorScalarPtr requires fp32 scalars)
            scal04 = pp.tile([P, 9], F32)
            nc.vector.tensor_scalar(scal04[:], scal[:, NREL:NREL + 9], 0.4,
                                    None, OP.mult)

            def w_ap(r):
                return scal[:, r:r + 1]

            s_ap = scal[:, 16:17]

            def c04_ap(i, j):
                return scal04[:, 3 * i + j:3 * i + j + 1]

            # scaled identities for the PE merged accumulation
            identw = []
            for r in range(NREL):
                t = pp.tile([P, P], BF16, tag=f"idw{r}")
                nc.vector.tensor_scalar(t[:], identb[:], w_ap(r), None, OP.mult)
                identw.append(t)
            # s-scaled identity: folds final_A = merged + s*tanh into the
            # transpose accumulation
            sidentb = pp.tile([P, P], BF16)
            nc.vector.tensor_scalar(sidentb[:], identb[:], s_ap, None, OP.mult)

            # ---- persistent big tensors (bf16; PSUM accumulates fp32) ----
            FT = pp.tile([P, KT * ROWS], BF16)    # final_A^T: 32 k-tiles x [128, 512]
            YG = pp.tile([P, KT * 2 * OUT], BF16)  # [Y1 | G] per k-tile

            # ---- DRAM bounce buffers (per-row-tile split collectives) ----
            sendb = [dpool.tile([NCORES * P, DST], BF16, tag=f"snd{i}",
                                name=f"sendb{i}")
                     for i in range(NT)]
            recvb = [dpool.tile([NCORES * P, DST], BF16, tag=f"rcv{i}",
                                name=f"recvb{i}")
                     for i in range(NT)]
            agin = dpool.tile([ROWS, OUT], BF16)
            agout = dpool.tile([N, OUT], BF16, addr_space="Shared")

            # ---- prep: Y1 = feature @ W1 ; G = feature @ (W1 W2) ----
            with (
                tc.tile_pool(name="prep", bufs=1) as prep,
                tc.tile_pool(name="preppsum", bufs=2, space="PSUM") as prps,
            ):
                fbf = prep.tile([NFEAT, N], BF16)
                nc.sync.dma_start(out=fbf[:], in_=featT[:])
                W1b = pp.tile([NFEAT, OUT], BF16)
                nc.vector.tensor_copy(W1b[:], W1t[:])
                W2b = pp.tile([OUT, OUT], BF16)
                nc.vector.tensor_copy(W2b[:], W2t[:])
                # W12 = W1 @ W2 (via W1^T transpose), h = b1 @ W2
                pw1t = prps.tile([P, P], BF16, tag="prsm")
                nc.tensor.transpose(pw1t[:OUT, :NFEAT], W1b[:], identb[:])
                W1T = prep.tile([OUT, NFEAT], BF16)
                nc.vector.tensor_copy(W1T[:], pw1t[:OUT, :NFEAT])
                pw12 = prps.tile([NFEAT, OUT], F32, tag="prsm")
                nc.tensor.matmul(pw12[:], lhsT=W1T[:], rhs=W2b[:],
                                 start=True, stop=True)
                W12b = pp.tile([NFEAT, OUT], BF16)
                nc.vector.tensor_copy(W12b[:], pw12[:])
                b1v = prep.tile([OUT, 1], BF16)
                pb1t = prps.tile([OUT, 1], BF16, tag="prsm")
                b1bf = prep.tile([1, OUT], BF16)
                nc.vector.tensor_copy(b1bf[:], b1st[:])
                nc.tensor.transpose(pb1t[:], b1bf[:], identb[:1, :1])
                nc.vector.tensor_copy(b1v[:], pb1t[:])
                phh = prps.tile([1, OUT], F32, tag="prsm")
                nc.tensor.matmul(phh[:], lhsT=b1v[:], rhs=W2b[:],
                                 start=True, stop=True)
                hst = prep.tile([1, OUT], F32)
                nc.vector.tensor_copy(hst[:], phh[:])
                phb = prps.tile([P, OUT], F32, tag="prsm")
                nc.tensor.matmul(phb[:], lhsT=ones_1p[:], rhs=hst[:],
                                 start=True, stop=True)
                hbf = pp.tile([P, OUT], BF16)
                nc.vector.tensor_copy(hbf[:], phb[:])
                hb32 = pp.tile([P, KT * OUT], BF16)
                for kt in range(KT):
                    nc.vector.tensor_copy(hb32[:, kt * OUT:(kt + 1) * OUT],
                                          hbf[:])

                for kt in range(KT):
                    pm = prps.tile([P, 2 * OUT], F32, tag="y1p")
                    nc.tensor.matmul(pm[:, :OUT],
                                     lhsT=fbf[:, kt * P:(kt + 1) * P],
                                     rhs=W1b[:], start=True, stop=True)
                    nc.tensor.matmul(pm[:, OUT:],
                                     lhsT=fbf[:, kt * P:(kt + 1) * P],
                                     rhs=W12b[:], start=True, stop=True)
                    nc.vector.tensor_copy(
                        YG[:, kt * 2 * OUT:(kt + 1) * 2 * OUT], pm[:])

            # ---- phase 1: stream A row block ----
            with (
                tc.tile_pool(name="rstr", bufs=2) as prr,
                tc.tile_pool(name="istr", bufs=2) as pis,
                tc.tile_pool(name="mstr", bufs=2) as pms,
                tc.tile_pool(name="mpsum", bufs=1, space="PSUM") as mps,
                tc.tile_pool(name="tpsum", bufs=4, space="PSUM") as tps,
            ):
                # transposes/copies run one chunk behind so the DVE never
                # stalls on the tanh -> PE -> copy chain of the same chunk
                pending = []

                def flush_pending():
                    # FT is i-major: [:, (i*KT + kt)*P], so 4 consecutive
                    # k-tiles share one wide PSUM tile and one wide copy
                    for (pi, pq, pmbf, ptT) in pending:
                        pc0 = pq * C
                        for g in range(C // P // 4):
                            kt0 = pc0 // P + g * 4
                            pt1 = tps.tile([P, 4 * P], F32, tag="ptg")
                            for t4 in range(4):
                                t = g * 4 + t4
                                sl = pt1[:, t4 * P:(t4 + 1) * P]
                                nc.tensor.matmul(
                                    sl, lhsT=pmbf[:, t * P:(t + 1) * P],
                                    rhs=identb[:], start=True, stop=False)
                                nc.tensor.matmul(
                                    sl, lhsT=ptT[:, t * P:(t + 1) * P],
                                    rhs=sidentb[:], start=False, stop=True)
                            fsl = FT[:, (pi * KT + kt0) * P:
                                     (pi * KT + kt0 + 4) * P]
                            nc.vector.tensor_copy(fsl, pt1[:])
                    pending.clear()

                for i in range(NT):
                    for q in range(NCH):
                        c0 = q * C
                        rb = []
                        for r in range(NREL):
                            rt = prr.tile([P, C], BF16, tag=f"r{r}")
                            nc.sync.dma_start(
                                out=rt[:],
                                in_=a_strip[r, i * P:(i + 1) * P, c0:c0 + C])
                            rb.append(rt)

                        # merged on PE: psum[b] = sum_r w_r R_r[:, b*512:...]
                        mpt = []
                        for b in range(4):
                            mp = mps.tile([P, 512], F32, tag=f"mp{b}")
                            for r in range(NREL):
                                nc.tensor.matmul(
                                    mp[:], lhsT=identw[r][:],
                                    rhs=rb[r][:, b * 512:(b + 1) * 512],
                                    start=(r == 0), stop=(r == NREL - 1))
                            mpt.append(mp)
                        # cast merged to bf16 (GpSimd cannot read PSUM) + send
                        mbf = pms.tile([P, C], BF16, tag="mbf")
                        for b in range(4):
                            nc.vector.tensor_copy(
                                mbf[:, b * 512:(b + 1) * 512], mpt[b][:])
                            d = q * 4 + b
                            nc.sync.dma_start(
                                out=sendb[i][d * P:(d + 1) * P, :],
                                in_=mbf[:, b * 512:(b + 1) * 512])

                        # interaction: acc = sum_j Q_j E_j
                        #   Q_j = 1.5 R_j + sign(R_j), E_j = sum 0.4 M_jo R_o
                        acc = pis.tile([P, C], BF16, tag="acc")
                        for j in range(3):
                            o1, o2 = [x for x in range(3) if x != j]
                            ta = pis.tile([P, C], BF16, tag="ta")
                            nc.scalar.activation(ta[:], rb[o1][:], AF.Copy,
                                                 scale=c04_ap(j, o1))
                            tb = pis.tile([P, C], BF16, tag="tb")
                            if j == 0:
                                nc.scalar.activation(tb[:], rb[o2][:], AF.Copy,
                                                     scale=c04_ap(j, o2))
                            else:
                                nc.vector.tensor_scalar(tb[:], rb[o2][:],
                                                        c04_ap(j, o2), None,
                                                        OP.mult)
                            ej = pis.tile([P, C], BF16, tag="ej")
                            nc.vector.tensor_tensor(ej[:], ta[:], tb[:], OP.add)
                            sg = pis.tile([P, C], BF16, tag="sg")
                            nc.scalar.sign(sg[:], rb[j][:])
                            qs = pis.tile([P, C], BF16, tag="qs")
                            nc.vector.tensor_scalar(qs[:], rb[j][:], 1.5,
                                                    None, OP.mult)
                            qj = pis.tile([P, C], BF16, tag="qj")
                            nc.vector.tensor_tensor(qj[:], qs[:], sg[:], OP.add)
                            if j == 0:
                                nc.vector.tensor_tensor(acc[:], qj[:], ej[:],
                                                        OP.mult)
                            else:
                                pj = pis.tile([P, C], BF16, tag="pj")
                                nc.vector.tensor_tensor(pj[:], qj[:], ej[:],
                                                        OP.mult)
                                nc.gpsimd.tensor_tensor(acc[:], acc[:],
                                                        pj[:], OP.add)
                        tT = pis.tile([P, C], BF16, tag="tT")
                        nc.scalar.activation(tT[:], acc[:], AF.Tanh)

                        flush_pending()
                        pending.append((i, q, mbf, tT))

                    # row tile i complete -> alltoall its merged rows
                    # (sends only need mbf, which is never deferred)
                    nc.gpsimd.collective_compute(
                        "AllToAll", OP.bypass, replica_groups=groups,
                        ins=[sendb[i][:].opt()], outs=[recvb[i][:].opt()])
                flush_pending()

            # ---- phase 2 ----
            with (
                tc.tile_pool(name="post", bufs=1) as post,
                tc.tile_pool(name="rcvp", bufs=4) as prc,
                tc.tile_pool(name="postpsum", bufs=1, space="PSUM") as pops,
                tc.tile_pool(name="u1psum", bufs=2, space="PSUM") as pup,
                tc.tile_pool(name="u1vpsum", bufs=1, space="PSUM") as pvac,
            ):
                # ---- stripe-major: FT += recv#j, then V and U1 matmuls
                # for stripe j, so stripes 0-2 overlap the last alltoall.
                # V and U1 accumulate separately so the V path can finish
                # (and the allgather launch) before U1 wraps up. ----
                pV = [pvac.tile([P, OUT], F32, tag=f"pV{i}",
                                name=f"pV{i}") for i in range(NT)]
                U1sb = post.tile([P, NT * OUT], F32, name="U1sb")

                def stripe_adds(j):
                    rt = prc.tile([P, NCORES * DST], BF16, tag="rt", bufs=2)
                    nc.sync.dma_start(
                        out=rt[:],
                        in_=recvb[j][:].rearrange("(d p) m -> p d m", p=P))
                    for d in range(NCORES):
                        kt = d * NT + j
                        for i in range(NT):
                            fsl = FT[:, (i * KT + kt) * P:(i * KT + kt + 1) * P]
                            nc.vector.tensor_tensor(
                                fsl, fsl,
                                rt[:, d * DST + i * P:d * DST + (i + 1) * P],
                                OP.add)

                def stripe_mm_v(j):
                    # V: one psum accumulation chain per row tile, held
                    # across all four stripes (one bank each)
                    for i in range(NT):
                        for d in range(NCORES):
                            kt = d * NT + j
                            nc.tensor.matmul(
                                pV[i][:],
                                lhsT=FT[:, (i * KT + kt) * P:
                                        (i * KT + kt + 1) * P],
                                rhs=YG[:, kt * 2 * OUT + OUT:
                                        (kt + 1) * 2 * OUT],
                                start=(j == 0 and d == 0),
                                stop=(j == NT - 1 and d == NCORES - 1))

                def stripe_mm_u(j):
                    # U1: per-stripe psum chain, accumulated into U1sb (SBUF)
                    for i in range(NT):
                        pu = pup.tile([P, OUT], F32, tag="pUji")
                        for d in range(NCORES):
                            kt = d * NT + j
                            nc.tensor.matmul(
                                pu[:],
                                lhsT=FT[:, (i * KT + kt) * P:
                                        (i * KT + kt + 1) * P],
                                rhs=YG[:, kt * 2 * OUT:kt * 2 * OUT + OUT],
                                start=(d == 0), stop=(d == NCORES - 1))
                        usl = U1sb[:, i * OUT:(i + 1) * OUT]
                        if j == 0:
                            nc.vector.tensor_tensor(usl, pu[:], b1b[:], OP.add)
                        else:
                            nc.vector.tensor_tensor(usl, usl, pu[:], OP.add)

                for j in range(NT - 1):
                    stripe_adds(j)
                    stripe_mm_v(j)
                    stripe_mm_u(j)

                # ---- struct branch (rank-7), overlaps alltoall tail ----
                encsb = post.tile([P, KT * NREL], F32)
                for kt in range(KT):
                    nc.sync.dma_start(out=encsb[:, kt * NREL:(kt + 1) * NREL],
                                      in_=encode[kt * P:(kt + 1) * P, :])
                encb = post.tile([P, KT * NREL], BF16)
                nc.vector.tensor_copy(encb[:], encsb[:])
                encT = post.tile([NREL, N], BF16)
                for kt in range(KT):
                    pte = pops.tile([P, P], BF16, tag="pp_tr")
                    nc.tensor.transpose(pte[:NREL, :],
                                        encb[:, kt * NREL:(kt + 1) * NREL],
                                        identb[:])
                    nc.vector.tensor_copy(encT[:, kt * P:(kt + 1) * P],
                                          pte[:NREL, :])
                encRsb = post.tile([P, NT * NREL], F32)
                for i in range(NT):
                    nc.sync.dma_start(out=encRsb[:, i * NREL:(i + 1) * NREL],
                                      in_=enc_rows[i * P:(i + 1) * P, :])
                encRb = post.tile([P, NT * NREL], BF16)
                nc.vector.tensor_copy(encRb[:], encRsb[:])
                encRT = post.tile([NREL, ROWS], BF16)
                for i in range(NT):
                    pte = pops.tile([P, P], BF16, tag="pp_tr")
                    nc.tensor.transpose(pte[:NREL, :],
                                        encRb[:, i * NREL:(i + 1) * NREL],
                                        identb[:])
                    nc.vector.tensor_copy(encRT[:, i * P:(i + 1) * P],
                                          pte[:NREL, :])

                # H1 = encode^T @ Y1, scaled by sw
                ph = pops.tile([NREL, OUT], F32, tag="pp_mm")
                for kt in range(KT):
                    nc.tensor.matmul(ph[:],
                                     lhsT=encb[:, kt * NREL:(kt + 1) * NREL],
                                     rhs=YG[:, kt * 2 * OUT:kt * 2 * OUT + OUT],
                                     start=(kt == 0), stop=(kt == KT - 1))
                H1p = post.tile([NREL, OUT], BF16)
                nc.scalar.activation(H1p[:], ph[:], AF.Copy, scale=swt[:])

                # U3 = encode @ H1p + b1 (full, replicated)
                U3sb = post.tile([P, KT * OUT], BF16)
                for kt in range(KT):
                    pm3 = pops.tile([P, OUT], F32, tag="pp_mm")
                    nc.tensor.matmul(pm3[:], lhsT=encT[:, kt * P:(kt + 1) * P],
                                     rhs=H1p[:], start=True, stop=True)
                    nc.vector.tensor_tensor(U3sb[:, kt * OUT:(kt + 1) * OUT],
                                            pm3[:], b1b[:], OP.add)

                # ---- final stripe: V path first so the allgather launches
                # as early as possible; U1 and struct part B fill its shadow
                stripe_adds(NT - 1)
                stripe_mm_v(NT - 1)
                for i in range(NT):
                    vt = post.tile([P, OUT], BF16, tag="vt", bufs=2)
                    nc.vector.tensor_copy(vt[:], pV[i][:])
                    nc.sync.dma_start(out=agin[i * P:(i + 1) * P, :], in_=vt[:])
                nc.gpsimd.collective_compute(
                    "AllGather", OP.bypass, replica_groups=groups,
                    ins=[agin[:].opt()], outs=[agout[:].opt()])
                stripe_mm_u(NT - 1)

                # G2 = (encode^T @ U3) @ W2, scaled by sw
                pg = pops.tile([NREL, OUT], F32, tag="pp_mm")
                for kt in range(KT):
                    nc.tensor.matmul(pg[:],
                                     lhsT=encb[:, kt * NREL:(kt + 1) * NREL],
                                     rhs=U3sb[:, kt * OUT:(kt + 1) * OUT],
                                     start=(kt == 0), stop=(kt == KT - 1))
                Gsb = post.tile([NREL, OUT], BF16)
                nc.vector.tensor_copy(Gsb[:], pg[:])
                pgt = pops.tile([OUT, NREL], BF16, tag="pp_tr")
                nc.tensor.transpose(pgt[:], Gsb[:], identb[:NREL, :NREL])
                GT = post.tile([OUT, NREL], BF16)
                nc.vector.tensor_copy(GT[:], pgt[:])
                pg2 = pops.tile([NREL, OUT], F32, tag="pp_mm")
                nc.tensor.matmul(pg2[:], lhsT=GT[:], rhs=W2b[:],
                                 start=True, stop=True)
                G2p = post.tile([NREL, OUT], BF16)
                nc.scalar.activation(G2p[:], pg2[:], AF.Copy, scale=swt[:])

                # U4 rows = enc_rows @ G2p + b2
                U4sb = post.tile([P, NT * OUT], F32)
                for i in range(NT):
                    pm4 = pops.tile([P, OUT], F32, tag="pp_mm")
                    nc.tensor.matmul(pm4[:], lhsT=encRT[:, i * P:(i + 1) * P],
                                     rhs=G2p[:], start=True, stop=True)
                    nc.vector.tensor_tensor(U4sb[:, i * OUT:(i + 1) * OUT],
                                            pm4[:], b2b[:], OP.add)

                # ---- Y2 = V_full + b1@W2 (one wide strided DMA) ----
                Y2 = post.tile([P, KT * OUT], BF16)
                Y2v = post.tile([P, KT * OUT], BF16)
                nc.sync.dma_start(
                    out=Y2v[:],
                    in_=agout[:].rearrange("(k p) o -> p k o", p=P))
                nc.vector.tensor_tensor(Y2[:], Y2v[:], hb32[:], OP.add)

                # ---- layer 2: U2 = final_A @ Y2 + b2 ----
                U2sb = post.tile([P, NT * OUT], F32)
                for i in range(NT):
                    pm = pops.tile([P, OUT], F32, tag="pp_mm")
                    for kt in range(KT):
                        nc.tensor.matmul(
                            pm[:],
                            lhsT=FT[:, (i * KT + kt) * P:(i * KT + kt + 1) * P],
                            rhs=Y2[:, kt * OUT:(kt + 1) * OUT],
                            start=(kt == 0), stop=(kt == KT - 1))
                    nc.vector.tensor_tensor(U2sb[:, i * OUT:(i + 1) * OUT],
                                            pm[:], b2b[:], OP.add)

                    # combine + normalize + store, interleaved per row tile
                    sl = slice(i * OUT, (i + 1) * OUT)
                    br1 = post.tile([P, OUT], F32, tag="br1", bufs=2)
                    nc.vector.tensor_tensor(br1[:], U1sb[:, sl], U2sb[:, sl],
                                            OP.add)
                    nc.vector.tensor_scalar(br1[:], br1[:], 0.5, None, OP.mult)
                    res = post.tile([P, OUT], F32, tag="res", bufs=2)
                    nc.vector.tensor_tensor(res[:], br1[:], U4sb[:, sl], OP.add)
                    nc.vector.tensor_scalar(res[:], res[:], 0.5, None, OP.mult)
                    _normalize(nc, post, pops, res, o_res, i)
                    _normalize(nc, post, pops, br1, o_b1, i)
                    u4 = post.tile([P, OUT], F32, tag="u4n", bufs=2)
                    nc.vector.tensor_copy(u4[:], U4sb[:, sl])
                    _normalize(nc, post, pops, u4, o_b2, i)

    _split_multi_waits(nc)
    return nc


_NC_CACHE = None


def get_nc():
    global _NC_CACHE
    if _NC_CACHE is None:
        _NC_CACHE = build_nc()
    return _NC_CACHE


def make_in_maps(feature, A_stack, encode, W1, b1, W2, b2, weight_b,
                 relation_interaction, interaction_strength, struct_weight):
    import ml_dtypes
    f32 = lambda x: np.ascontiguousarray(np.asarray(x, dtype=np.float32))
    featT = np.ascontiguousarray(
        np.asarray(feature, np.float32).T.astype(ml_dtypes.bfloat16))
    enc = f32(encode)
    common = dict(
        featT=featT,
        encode=enc,
        W1=f32(W1),
        W2=f32(W2),
        b1=f32(np.reshape(b1, (1, OUT))),
        b2=f32(np.reshape(b2, (1, OUT))),
        wb=f32(np.reshape(np.asarray(weight_b, np.float32)[:, 0], (1, NREL))),
        ri=f32(np.reshape(relation_interaction, (1, 9))),
        s_=f32(np.reshape(interaction_strength, (1, 1))),
        sw=f32(np.reshape(struct_weight, (NREL, 1))),
    )
    in_maps = []
    A = np.asarray(A_stack, np.float32).astype(ml_dtypes.bfloat16)
    for c in range(NCORES):
        rows = slice(c * ROWS, (c + 1) * ROWS)
        m = dict(common)
        m["a_strip"] = np.ascontiguousarray(A[:, rows, :])
        m["enc_rows"] = f32(enc[rows])
        in_maps.append(m)
    return in_maps


def run(inputs, trace=False, tmpdir=None):
    nc = get_nc()
    in_maps = make_in_maps(**inputs)
    kres = run_bass_kernel_spmd(nc, in_maps, list(range(NCORES)),
                                trace=trace, tmpdir=tmpdir)
    res = kres.results
    result = np.concatenate([res[c]["o_res"] for c in range(NCORES)], axis=0)
    branch1 = np.concatenate([res[c]["o_b1"] for c in range(NCORES)], axis=0)
    branch2 = np.concatenate([res[c]["o_b2"] for c in range(NCORES)], axis=0)
    return (result, branch1, branch2), kres


def kernel(**inputs):
    return run(inputs)[0]
